# revision 5
# baseline (speedup 1.0000x reference)
"""Distributed single-head attention block for trn2 (8 NeuronCores), v2.

reference:
    q = x @ Wq.T + bq ; k = x @ Wk.T + bk ; v = x @ Wv.T + bv
    out = x + softmax(q @ k.T / sqrt(D)) @ v       x: [4, 2048, 1024]

Sharding: 8 cores = 4 batches x 2 halves. Core c owns batch c//2 and
rows [h*1024, (h+1)*1024) with h = c%2 — both as queries and as keys.
Each core projects Q, K and V only for its OWN half (perfectly balanced
3x1024x1024x1024 MACs of projection per core) and swaps the K/V halves
with its pair partner. Local key order is [own half | partner half];
attention is permutation-invariant over keys so the order never needs
to be undone.

Partner exchange without per-core code divergence: the SPMD graph is
identical on all cores, so "read the partner's AllGather slot" (slot
1-h) is not expressible. Instead each core stages TWO masked copies of
its own half — slot s gets own*m[s] with host-supplied masks
m = [h, 1-h] — and a pairwise ReduceScatter(add) then delivers
exactly the partner's half (partner + 0) into a fixed output buffer.
Own halves never round-trip through the collective, so own-half scores
and attention chunks have no collective dependency at all.

Device-side layouts (host pre-transposes + bf16-casts so the
contraction dim always lands on SBUF partitions):
    xqT  [D, SQ]            bf16  x[b, half].T   -> Q/K/V projections
    xq   [SQ, D]            f32   x[b, half]+bv  -> residual add
    wvT  [D, D]             bf16  Wv.T
    wkE  [EC, 128, DC, 128] bf16  Wk.T e-chunk-major (wkE[e c ,p,dc,j]
          = Wk.T[dc*128+p, ec*128+j]) so the first kT chain only needs
          xqT + one 256KB e-chunk: first real matmul at ~7us.
    wqE  same for Wq.T
Projections emit qT/kT [e, s] (scores contraction over e) and v [s, e]
(attn contraction over keys). Softmax rows live on partitions: exp on
ScalarE with accum_out giving row sums for free; no max subtraction
(scores are O(10) for this model so exp cannot overflow in f32). P is
transposed 128x128 on TensorE (identity matmul) for the attn matmul.
Scores run kc-outer (key-chunk-major) so the partner-K chunks are the
last PE work enqueued before attention — maximum slack for the
ReduceScatter. Only the Q bias is applied on-device: the K bias
cancels in softmax (per-row constant), and the V bias is folded into
the residual input on the host (attention weights sum to 1).
"""

import numpy as np

B, S, D = 4, 2048, 1024
SQ = S // 2  # queries/keys owned per core
NCORES = 8
DC = D // 128  # contraction chunks
EC = D // 128  # embed chunks
SC = S // 128  # key chunks, full batch
SCH = SQ // 128  # key chunks per half
QT = SQ // 128  # query tiles per core
KC = S // 512  # score column chunks, full batch
KCH = SQ // 512  # score column chunks per half

_cache = {}


def _build():
    import concourse.bass as bass
    import concourse.tile as tile
    from concourse import bacc, mybir
    from concourse.masks import make_identity

    f32 = mybir.dt.float32
    bf16 = mybir.dt.bfloat16
    Alu = mybir.AluOpType
    Act = mybir.ActivationFunctionType

    nc = bacc.Bacc(None, target_bir_lowering=False, debug=False)

    xqT_d = nc.declare_dram_parameter("xqT", [D, SQ], bf16, isOutput=False)
    xq_d = nc.declare_dram_parameter("xq", [SQ, D], f32, isOutput=False)
    wqE_d = nc.declare_dram_parameter("wqE", [EC, 128, DC, 128], bf16, isOutput=False)
    wkE_d = nc.declare_dram_parameter("wkE", [EC, 128, DC, 128], bf16, isOutput=False)
    wv_d = nc.declare_dram_parameter("wvT", [D, D], bf16, isOutput=False)
    bq_d = nc.declare_dram_parameter("bq", [D], f32, isOutput=False)
    m_d = nc.declare_dram_parameter("m", [128, 2], f32, isOutput=False)
    out_d = nc.declare_dram_parameter("out", [SQ, D], f32, isOutput=True)

    # Pairwise exchange staging. Slot s of *_in holds own*m[s]; the
    # ReduceScatter(add) over [[0,1],...] leaves partner's half in *_out.
    kx_in = nc.dram_tensor("kx_in", [2, EC, KCH, 128, 512], bf16)
    kx_out = nc.dram_tensor("kx_out", [EC, KCH, 128, 512], bf16)
    vx_in = nc.dram_tensor("vx_in", [2, SCH, 2, 128, 512], bf16)
    vx_out = nc.dram_tensor("vx_out", [SCH, 2, 128, 512], bf16)

    groups = [[0, 1], [2, 3], [4, 5], [6, 7]]

    with tile.TileContext(nc) as tc:
        with tc.tile_pool(name="pers", bufs=1) as pers:
            qT_sb = pers.tile([128, EC, SQ], bf16, tag="qT")
            kT_sb = pers.tile([128, EC, S], bf16, tag="kT")
            v_sb = pers.tile([128, SC, D], bf16, tag="v")
            ident = pers.tile([128, 128], bf16, tag="ident")
            make_identity(nc, ident)
            bq_sb = pers.tile([128, EC], f32, tag="bq")
            m_sb = pers.tile([128, 2], f32, tag="m")

            # PE warmup: dense dummy matmuls while the first input DMAs land,
            # so the HAM clock gate is already at 2.4GHz when real work starts.
            warm_sb = pers.tile([128, 512], bf16, tag="warm")
            warm_dump = pers.tile([128, 512], f32, tag="warm_dump")
            nc.vector.memset(warm_sb, 0.0)
            with tc.tile_pool(name="warm_ps", bufs=1, space="PSUM") as warm_ps:
                wps = warm_ps.tile([128, 512], f32, tag="wps")
                NWARM = 14
                for i in range(NWARM):
                    nc.tensor.matmul(
                        wps,
                        lhsT=warm_sb[:, 0:128],
                        rhs=warm_sb,
                        start=(i == 0),
                        stop=(i == NWARM - 1),
                    )
                nc.vector.tensor_copy(out=warm_dump, in_=wps)

            bq_ap = bq_d.ap()
            nc.scalar.dma_start(
                out=bq_sb,
                in_=bass.AP(tensor=bq_ap.tensor, offset=0, ap=[[1, 128], [128, EC]]),
            )
            nc.scalar.dma_start(out=m_sb, in_=m_d[:, :])

            with (
                tc.tile_pool(name="ld", bufs=1) as ld,
                tc.tile_pool(name="stage", bufs=6) as stage,
                tc.tile_pool(name="proj_ps", bufs=4, space="PSUM") as proj_ps,
            ):
                xqT_sb = ld.tile([128, DC, SQ], bf16, tag="xqT")
                wk_sb = ld.tile([128, EC, DC, 128], bf16, tag="wk")
                wq_sb = ld.tile([128, EC, DC, 128], bf16, tag="wq")
                wv_sb = ld.tile([128, DC, D], bf16, tag="wv")

                # DMA priority: xqT gates every projection; wk e-chunks gate
                # the kT chains (and with them the K exchange); then wv, wq.
                for dc in range(DC):
                    r = slice(dc * 128, (dc + 1) * 128)
                    nc.sync.dma_start(out=xqT_sb[:, dc, :], in_=xqT_d[r, :])
                for ec in range(EC):
                    nc.sync.dma_start(out=wk_sb[:, ec, :, :], in_=wkE_d[ec])
                for dc in range(DC):
                    r = slice(dc * 128, (dc + 1) * 128)
                    nc.sync.dma_start(out=wv_sb[:, dc, :], in_=wv_d[r, :])
                for ec in range(EC):
                    nc.sync.dma_start(out=wq_sb[:, ec, :, :], in_=wqE_d[ec])

                # kT own half [e, sk_own]: feed own region of kT_sb directly
                # and stage the two masked copies for the exchange.
                for ec in range(EC):
                    for kc in range(KCH):
                        csl = slice(kc * 512, (kc + 1) * 512)
                        ps = proj_ps.tile([128, 512], f32, tag="ps")
                        for dc in range(DC):
                            nc.tensor.matmul(
                                ps,
                                lhsT=wk_sb[:, ec, dc, :],
                                rhs=xqT_sb[:, dc, csl],
                                start=(dc == 0),
                                stop=(dc == DC - 1),
                            )
                        nc.vector.tensor_copy(out=kT_sb[:, ec, csl], in_=ps)
                        st0 = stage.tile([128, 512], bf16, tag="st")
                        st1 = stage.tile([128, 512], bf16, tag="st")
                        nc.vector.tensor_scalar_mul(
                            out=st0, in0=ps, scalar1=m_sb[:, 0:1]
                        )
                        nc.vector.tensor_scalar_mul(
                            out=st1, in0=ps, scalar1=m_sb[:, 1:2]
                        )
                        nc.scalar.dma_start(out=kx_in[0, ec, kc], in_=st0)
                        nc.scalar.dma_start(out=kx_in[1, ec, kc], in_=st1)
                nc.gpsimd.collective_compute(
                    "ReduceScatter",
                    Alu.add,
                    replica_groups=groups,
                    ins=[kx_in.ap().opt()],
                    outs=[kx_out.ap().opt()],
                )

                # v own half [sk_own, e], same pattern.
                for sc in range(SCH):
                    for j in range(D // 512):
                        jsl = slice(j * 512, (j + 1) * 512)
                        ps = proj_ps.tile([128, 512], f32, tag="ps")
                        for dc in range(DC):
                            nc.tensor.matmul(
                                ps,
                                lhsT=xqT_sb[:, dc, sc * 128 : (sc + 1) * 128],
                                rhs=wv_sb[:, dc, jsl],
                                start=(dc == 0),
                                stop=(dc == DC - 1),
                            )
                        nc.vector.tensor_copy(out=v_sb[:, sc, jsl], in_=ps)
                        st0 = stage.tile([128, 512], bf16, tag="st")
                        st1 = stage.tile([128, 512], bf16, tag="st")
                        nc.vector.tensor_scalar_mul(
                            out=st0, in0=ps, scalar1=m_sb[:, 0:1]
                        )
                        nc.vector.tensor_scalar_mul(
                            out=st1, in0=ps, scalar1=m_sb[:, 1:2]
                        )
                        nc.scalar.dma_start(out=vx_in[0, sc, j], in_=st0)
                        nc.scalar.dma_start(out=vx_in[1, sc, j], in_=st1)
                nc.gpsimd.collective_compute(
                    "ReduceScatter",
                    Alu.add,
                    replica_groups=groups,
                    ins=[vx_in.ap().opt()],
                    outs=[vx_out.ap().opt()],
                )

                # Readbacks AFTER both stage-out loops: the scalar DMA queue
                # is in-order, and rb-k waits on the K collective — putting
                # it ahead of the V stage-outs would stall the V exchange.
                for ec in range(EC):
                    for kc in range(KCH):
                        nc.scalar.dma_start(
                            out=kT_sb[:, ec, SQ + kc * 512 : SQ + (kc + 1) * 512],
                            in_=kx_out[ec, kc],
                        )
                for sc in range(SCH):
                    for j in range(D // 512):
                        nc.scalar.dma_start(
                            out=v_sb[:, SCH + sc, j * 512 : (j + 1) * 512],
                            in_=vx_out[sc, j],
                        )

                # qT[e, sq] = sum_d wqT[d, e] * xqT[d, sq]  (+bq per-partition)
                for j in range(SQ // 512):
                    jsl = slice(j * 512, (j + 1) * 512)
                    for ec in range(EC):
                        ps = proj_ps.tile([128, 512], f32, tag="ps")
                        for dc in range(DC):
                            nc.tensor.matmul(
                                ps,
                                lhsT=wq_sb[:, ec, dc, :],
                                rhs=xqT_sb[:, dc, jsl],
                                start=(dc == 0),
                                stop=(dc == DC - 1),
                            )
                        nc.vector.tensor_scalar_add(
                            out=qT_sb[:, ec, jsl],
                            in0=ps,
                            scalar1=bq_sb[:, ec : ec + 1],
                        )

            with (
                tc.tile_pool(name="att", bufs=2) as att,
                tc.tile_pool(name="small", bufs=2) as small,
                tc.tile_pool(name="score_ps", bufs=3, space="PSUM") as score_ps,
                tc.tile_pool(name="tr_ps", bufs=2, space="PSUM") as tr_ps,
                tc.tile_pool(name="attn_ps", bufs=3, space="PSUM") as attn_ps,
            ):
                inv_sqrt_d = float(1.0 / np.sqrt(D))
                # pass 1: scores -> exp -> P transposes, kc-outer so the two
                # own-key chunks (no collective dep) are enqueued first and
                # the partner-K chunks last. The PE queue is in-order, so
                # this makes the kernel robust to a late K exchange; the attn
                # matmuls sit behind everything, covering a late V exchange.
                P_list = [
                    att.tile([128, S], bf16, name=f"P{qt}", tag=f"P{qt}", bufs=1)
                    for qt in range(QT)
                ]
                PT_list = [
                    att.tile(
                        [128, SC, 128], bf16, name=f"PT{qt}", tag=f"PT{qt}", bufs=1
                    )
                    for qt in range(QT)
                ]
                den4_list = [
                    small.tile([128, KC], f32, name=f"den4{qt}", tag=f"den4{qt}", bufs=1)
                    for qt in range(QT)
                ]
                recip_list = [
                    small.tile(
                        [128, 1], f32, name=f"recip{qt}", tag=f"recip{qt}", bufs=1
                    )
                    for qt in range(QT)
                ]
                for kc in range(KC):
                    csl = slice(kc * 512, (kc + 1) * 512)
                    for qt in range(QT):
                        qsl = slice(qt * 128, (qt + 1) * 128)
                        ps = score_ps.tile([128, 512], f32, tag="score")
                        for ec in range(EC):
                            nc.tensor.matmul(
                                ps,
                                lhsT=qT_sb[:, ec, qsl],
                                rhs=kT_sb[:, ec, csl],
                                start=(ec == 0),
                                stop=(ec == EC - 1),
                            )
                        nc.scalar.activation(
                            out=P_list[qt][:, csl],
                            in_=ps,
                            func=Act.Exp,
                            scale=inv_sqrt_d,
                            accum_out=den4_list[qt][:, kc : kc + 1],
                        )
                        for j in range(kc * 4, kc * 4 + 4):
                            tp = tr_ps.tile([128, 128], bf16, tag="tr")
                            nc.tensor.transpose(
                                tp, P_list[qt][:, j * 128 : (j + 1) * 128], ident
                            )
                            nc.vector.tensor_copy(out=PT_list[qt][:, j, :], in_=tp)
                for qt in range(QT):
                    den = small.tile([128, 1], f32, tag="den", bufs=4)
                    nc.vector.reduce_sum(
                        out=den, in_=den4_list[qt], axis=mybir.AxisListType.X
                    )
                    nc.vector.reciprocal(recip_list[qt], den)

                # pass 2: attn + epilogue per q-tile (partner-V lands mid-pass1)
                for qt in range(QT):
                    qsl = slice(qt * 128, (qt + 1) * 128)
                    PT_sb = PT_list[qt]
                    recip = recip_list[qt]
                    xq_sb = att.tile([128, D], f32, tag="xq", bufs=3)
                    nc.sync.dma_start(out=xq_sb, in_=xq_d[qsl, :])
                    ot = att.tile([128, D], f32, tag="ot", bufs=3)
                    for j2 in range(D // 512):
                        jsl = slice(j2 * 512, (j2 + 1) * 512)
                        pa = attn_ps.tile([128, 512], f32, tag="attn")
                        for j in range(SC):
                            nc.tensor.matmul(
                                pa,
                                lhsT=PT_sb[:, j, :],
                                rhs=v_sb[:, j, jsl],
                                start=(j == 0),
                                stop=(j == SC - 1),
                            )
                        # out = attn * (1/den) + residual
                        nc.vector.scalar_tensor_tensor(
                            out=ot[:, jsl],
                            in0=pa,
                            scalar=recip,
                            in1=xq_sb[:, jsl],
                            op0=Alu.mult,
                            op1=Alu.add,
                        )
                        nc.scalar.dma_start(out=out_d[qsl, jsl], in_=ot[:, jsl])

    nc.compile()
    return nc


def _get_nc():
    if "nc" not in _cache:
        _cache["nc"] = _build()
    return _cache["nc"]


def kernel(embedded, Wq, bq, Wk, bk, Wv, bv):
    import ml_dtypes

    from concourse.bass_utils import run_bass_kernel_spmd

    bf16 = ml_dtypes.bfloat16
    x = np.ascontiguousarray(np.asarray(embedded, dtype=np.float32))
    Wq = np.asarray(Wq, dtype=np.float32)
    Wk = np.asarray(Wk, dtype=np.float32)
    Wv = np.asarray(Wv, dtype=np.float32)
    bq = np.ascontiguousarray(np.asarray(bq, dtype=np.float32))
    bk = np.ascontiguousarray(np.asarray(bk, dtype=np.float32))
    bv = np.ascontiguousarray(np.asarray(bv, dtype=np.float32))

    # e-chunk-major weight layouts: wE[ec, p, dc, j] = W.T[dc*128+p, ec*128+j]
    def echunk(wT):
        return np.ascontiguousarray(
            wT.reshape(DC, 128, EC, 128).transpose(2, 1, 0, 3)
        )

    wqT = np.ascontiguousarray(Wq.T).astype(bf16)
    wkT = np.ascontiguousarray(Wk.T).astype(bf16)
    wvT = np.ascontiguousarray(Wv.T).astype(bf16)
    wqE = echunk(wqT)
    wkE = echunk(wkT)
    xT = [np.ascontiguousarray(x[b].T).astype(bf16) for b in range(B)]

    in_maps = []
    for c in range(NCORES):
        b, h = c // 2, c % 2
        qs = slice(h * SQ, (h + 1) * SQ)
        m = np.zeros((128, 2), dtype=np.float32)
        m[:, 0] = float(h)
        m[:, 1] = float(1 - h)
        in_maps.append(
            {
                "xqT": np.ascontiguousarray(xT[b][:, qs]),
                "xq": np.ascontiguousarray(x[b, qs, :] + bv),
                "wqE": wqE,
                "wkE": wkE,
                "wvT": wvT,
                "bq": bq,
                "m": m,
            }
        )

    _cache["in_maps"] = in_maps
    nc = _get_nc()
    res = run_bass_kernel_spmd(nc, in_maps, core_ids=list(range(NCORES)))
    out = np.empty((B, S, D), dtype=np.float32)
    for c in range(NCORES):
        b, h = c // 2, c % 2
        out[b, h * SQ : (h + 1) * SQ, :] = res.results[c]["out"]
    return out


# revision 6
# speedup vs baseline: 1.3119x; 1.3119x over previous
"""Distributed single-head attention block for trn2 (8 NeuronCores), v3.

reference:
    q = x @ Wq.T + bq ; k = x @ Wk.T + bk ; v = x @ Wv.T + bv
    out = x + softmax(q @ k.T / sqrt(D)) @ v       x: [4, 2048, 1024]

Sharding: 8 cores = 4 batches x 2 halves. Core c owns batch c//2 and
rows [h*1024, (h+1)*1024) with h = c%2 — both as queries and as keys.
Each core projects Q, K and V only for its OWN half (perfectly
balanced: 3 x 1024^3 MACs of projection per core vs 4 x 1024^3 in the
K-replicated variant) and the pair exchanges K/V halves with two
pairwise AllGathers (2MB each), landing in natural batch order.

Engine-queue discipline (learned from v2's trace): a dma_start costs
~0.7us of ISSUE time on the triggering engine's in-order queue, so DMA
triggers must never sit ahead of latency-critical compute on the same
queue, and a trigger that waits on a collective poisons everything
behind it:
    sync   : weight/activation input loads, then K/V slot-0 readbacks,
             then residual (xq) loads
    scalar : bq load + K/V stage-outs (early), exp activations, output
             stores
    vector : psum->SBUF copies/casts and all other DVE math only
    gpsimd : collective triggers, then K/V slot-1 readbacks
Scores wait only on the K readbacks (~91us, PE is busy until ~89us);
attention waits on the V readbacks (~121us, PE busy until ~150us).

Device-side layouts (host pre-transposes + bf16-casts so the
contraction dim always lands on SBUF partitions):
    xqT  [D, SQ]            bf16  x[b, half].T   -> Q/K/V projections
    xq   [SQ, D]            f32   x[b, half]+bv  -> residual add
    wvT  [D, D]             bf16  Wv.T
    wkE  [EC, 128, DC, 128] bf16  Wk.T e-chunk-major (wkE[ec,p,dc,j]
          = Wk.T[dc*128+p, ec*128+j]) so the first kT chain only needs
          xqT + one 256KB e-chunk: first real matmul at ~7us.
    wqE  same for Wq.T
Projections emit qT/kT [e, s] (scores contraction over e) and v [s, e]
(attn contraction over keys). Softmax rows live on partitions: exp on
ScalarE with accum_out giving row sums for free; no max subtraction
(scores are O(10) for this model so exp cannot overflow in f32). P is
transposed 128x128 on TensorE (identity matmul) for the attn matmul;
transposes are emitted TWO score-chains late so the PE never waits on
the exp that produces their input. Score order is slot-0 chunks for
all q-tiles first (own-slot readback lands first), then kc in {2,3}
per q-tile so each tile's softmax denominator (and its reciprocal) is
ready long before the attention epilogue needs it. Only the Q bias is
applied on-device: the K bias cancels in softmax (per-row constant),
and the V bias is folded into the residual input on the host
(attention weights sum to 1).
"""

import numpy as np

B, S, D = 4, 2048, 1024
SQ = S // 2  # queries/keys owned per core
NCORES = 8
DC = D // 128  # contraction chunks
EC = D // 128  # embed chunks
SC = S // 128  # key chunks, full batch
SCH = SQ // 128  # key chunks per half
QT = SQ // 128  # query tiles per core
KC = S // 512  # score column chunks, full batch
KCH = SQ // 512  # score column chunks per half
EJ = D // 512  # 512-wide embed column chunks

_cache = {}


def _build():
    import concourse.bass as bass
    import concourse.tile as tile
    from concourse import bacc, mybir
    from concourse.masks import make_identity

    f32 = mybir.dt.float32
    bf16 = mybir.dt.bfloat16
    Alu = mybir.AluOpType
    Act = mybir.ActivationFunctionType

    nc = bacc.Bacc(None, target_bir_lowering=False, debug=False)

    xqT_d = nc.declare_dram_parameter("xqT", [D, SQ], bf16, isOutput=False)
    xq_d = nc.declare_dram_parameter("xq", [SQ, D], f32, isOutput=False)
    wqE_d = nc.declare_dram_parameter("wqE", [EC, 128, DC, 128], bf16, isOutput=False)
    wkE_d = nc.declare_dram_parameter("wkE", [EC, 128, DC, 128], bf16, isOutput=False)
    wv_d = nc.declare_dram_parameter("wvT", [D, D], bf16, isOutput=False)
    bq_d = nc.declare_dram_parameter("bq", [D], f32, isOutput=False)
    out_d = nc.declare_dram_parameter("out", [SQ, D], f32, isOutput=True)

    # Pairwise K/V exchange staging; AllGather slot order = natural
    # batch order, identical on both pair members (uniform SPMD graph).
    kx_in = nc.dram_tensor("kx_in", [EC, KCH, 128, 512], bf16)
    kx_out = nc.dram_tensor("kx_out", [2, EC, KCH, 128, 512], bf16)
    vx_in = nc.dram_tensor("vx_in", [SCH, EJ, 128, 512], bf16)
    vx_out = nc.dram_tensor("vx_out", [2, SCH, EJ, 128, 512], bf16)

    groups = [[0, 1], [2, 3], [4, 5], [6, 7]]

    with tile.TileContext(nc) as tc:
        with tc.tile_pool(name="pers", bufs=1) as pers:
            qT_sb = pers.tile([128, EC, SQ], bf16, tag="qT")
            # AG-fed K/V: [slot, ec|sc, col-chunk, 512]
            kT_sb = pers.tile([128, 2, EC, KCH, 512], bf16, tag="kT")
            v_sb = pers.tile([128, 2, SCH, EJ, 512], bf16, tag="v")
            ident = pers.tile([128, 128], bf16, tag="ident")
            make_identity(nc, ident)
            bq_sb = pers.tile([128, EC], f32, tag="bq")

            # PE warmup: dense dummy matmuls while the first input DMAs land,
            # so the HAM clock gate is already at 2.4GHz when real work starts.
            warm_sb = pers.tile([128, 512], bf16, tag="warm")
            warm_dump = pers.tile([128, 512], f32, tag="warm_dump")
            nc.vector.memset(warm_sb, 0.0)
            with tc.tile_pool(name="warm_ps", bufs=1, space="PSUM") as warm_ps:
                wps = warm_ps.tile([128, 512], f32, tag="wps")
                NWARM = 14
                for i in range(NWARM):
                    nc.tensor.matmul(
                        wps,
                        lhsT=warm_sb[:, 0:128],
                        rhs=warm_sb,
                        start=(i == 0),
                        stop=(i == NWARM - 1),
                    )
                nc.vector.tensor_copy(out=warm_dump, in_=wps)

            bq_ap = bq_d.ap()
            nc.scalar.dma_start(
                out=bq_sb,
                in_=bass.AP(tensor=bq_ap.tensor, offset=0, ap=[[1, 128], [128, EC]]),
            )

            with (
                tc.tile_pool(name="ld", bufs=1) as ld,
                tc.tile_pool(name="stage", bufs=4) as stage,
                tc.tile_pool(name="proj_ps", bufs=4, space="PSUM") as proj_ps,
            ):
                xqT_sb = ld.tile([128, DC, SQ], bf16, tag="xqT")
                wk_sb = ld.tile([128, EC, DC, 128], bf16, tag="wk")
                wq_sb = ld.tile([128, EC, DC, 128], bf16, tag="wq")
                wv_sb = ld.tile([128, DC, D], bf16, tag="wv")

                # DMA priority: xqT gates every projection; wk e-chunks gate
                # the kT chains (and with them the K exchange); then wv, wq.
                for dc in range(DC):
                    r = slice(dc * 128, (dc + 1) * 128)
                    nc.sync.dma_start(out=xqT_sb[:, dc, :], in_=xqT_d[r, :])
                for ec in range(EC):
                    nc.sync.dma_start(out=wk_sb[:, ec, :, :], in_=wkE_d[ec])
                for dc in range(DC):
                    r = slice(dc * 128, (dc + 1) * 128)
                    nc.sync.dma_start(out=wv_sb[:, dc, :], in_=wv_d[r, :])
                for ec in range(EC):
                    nc.sync.dma_start(out=wq_sb[:, ec, :, :], in_=wqE_d[ec])

                # kT own half [e, sk_own] -> bf16 stage tiles -> DRAM
                for ec in range(EC):
                    for kc in range(KCH):
                        csl = slice(kc * 512, (kc + 1) * 512)
                        ps = proj_ps.tile([128, 512], f32, tag="ps")
                        for dc in range(DC):
                            nc.tensor.matmul(
                                ps,
                                lhsT=wk_sb[:, ec, dc, :],
                                rhs=xqT_sb[:, dc, csl],
                                start=(dc == 0),
                                stop=(dc == DC - 1),
                            )
                        kst = stage.tile([128, 512], bf16, tag="kst")
                        nc.vector.tensor_copy(out=kst, in_=ps)
                        nc.scalar.dma_start(out=kx_in[ec, kc], in_=kst)
                nc.gpsimd.collective_compute(
                    "AllGather",
                    Alu.bypass,
                    replica_groups=groups,
                    ins=[kx_in.ap().opt()],
                    outs=[kx_out.ap().opt()],
                )

                # v own half [sk_own, e], same pattern.
                for sc in range(SCH):
                    for j in range(EJ):
                        jsl = slice(j * 512, (j + 1) * 512)
                        ps = proj_ps.tile([128, 512], f32, tag="ps")
                        for dc in range(DC):
                            nc.tensor.matmul(
                                ps,
                                lhsT=xqT_sb[:, dc, sc * 128 : (sc + 1) * 128],
                                rhs=wv_sb[:, dc, jsl],
                                start=(dc == 0),
                                stop=(dc == DC - 1),
                            )
                        vst = stage.tile([128, 512], bf16, tag="vst")
                        nc.vector.tensor_copy(out=vst, in_=ps)
                        nc.scalar.dma_start(out=vx_in[sc, j], in_=vst)
                nc.gpsimd.collective_compute(
                    "AllGather",
                    Alu.bypass,
                    replica_groups=groups,
                    ins=[vx_in.ap().opt()],
                    outs=[vx_out.ap().opt()],
                )

                # Readbacks: slot 0 on the (now idle) sync queue, slot 1 on
                # gpsimd, K before V on each. Score chains consume slot 0
                # first, so slot-0 readbacks lead.
                for ec in range(EC):
                    for kc in range(KCH):
                        nc.sync.dma_start(
                            out=kT_sb[:, 0, ec, kc, :], in_=kx_out[0, ec, kc]
                        )
                        nc.gpsimd.dma_start(
                            out=kT_sb[:, 1, ec, kc, :], in_=kx_out[1, ec, kc]
                        )
                for sc in range(SCH):
                    for j in range(EJ):
                        nc.sync.dma_start(
                            out=v_sb[:, 0, sc, j, :], in_=vx_out[0, sc, j]
                        )
                        nc.gpsimd.dma_start(
                            out=v_sb[:, 1, sc, j, :], in_=vx_out[1, sc, j]
                        )

                # qT[e, sq] = sum_d wqT[d, e] * xqT[d, sq]  (+bq per-partition)
                for j in range(SQ // 512):
                    jsl = slice(j * 512, (j + 1) * 512)
                    for ec in range(EC):
                        ps = proj_ps.tile([128, 512], f32, tag="ps")
                        for dc in range(DC):
                            nc.tensor.matmul(
                                ps,
                                lhsT=wq_sb[:, ec, dc, :],
                                rhs=xqT_sb[:, dc, jsl],
                                start=(dc == 0),
                                stop=(dc == DC - 1),
                            )
                        nc.vector.tensor_scalar_add(
                            out=qT_sb[:, ec, jsl],
                            in0=ps,
                            scalar1=bq_sb[:, ec : ec + 1],
                        )

            with (
                tc.tile_pool(name="att", bufs=2) as att,
                tc.tile_pool(name="small", bufs=2) as small,
                tc.tile_pool(name="score_ps", bufs=3, space="PSUM") as score_ps,
                tc.tile_pool(name="tr_ps", bufs=3, space="PSUM") as tr_ps,
                tc.tile_pool(name="attn_ps", bufs=2, space="PSUM") as attn_ps,
            ):
                inv_sqrt_d = float(1.0 / np.sqrt(D))
                P_list = [
                    att.tile([128, S], bf16, name=f"P{qt}", tag=f"P{qt}", bufs=1)
                    for qt in range(QT)
                ]
                PT_list = [
                    att.tile(
                        [128, SC, 128], bf16, name=f"PT{qt}", tag=f"PT{qt}", bufs=1
                    )
                    for qt in range(QT)
                ]
                den4_list = [
                    small.tile([128, KC], f32, name=f"den4{qt}", tag=f"den4{qt}", bufs=1)
                    for qt in range(QT)
                ]
                recip_list = [
                    small.tile(
                        [128, 1], f32, name=f"recip{qt}", tag=f"recip{qt}", bufs=1
                    )
                    for qt in range(QT)
                ]

                # pass 1: scores -> exp -> P transposes. Unit order: slot-0
                # key chunks across all q-tiles first (their readback lands
                # first), then kc in {2,3} per q-tile so den/recip per tile
                # completes well before the attn epilogue. Transposes for a
                # unit are emitted two chains late so the PE never waits on
                # that unit's exp.
                units = [(kc, qt) for kc in range(KCH) for qt in range(QT)]
                units += [(kc, qt) for qt in range(QT) for kc in range(KCH, KC)]

                def emit_transposes(kc, qt):
                    for j in range(kc * 4, kc * 4 + 4):
                        tp = tr_ps.tile([128, 128], bf16, tag="tr")
                        nc.tensor.transpose(
                            tp, P_list[qt][:, j * 128 : (j + 1) * 128], ident
                        )
                        nc.vector.tensor_copy(out=PT_list[qt][:, j, :], in_=tp)

                for i, (kc, qt) in enumerate(units):
                    csl = slice(kc * 512, (kc + 1) * 512)
                    qsl = slice(qt * 128, (qt + 1) * 128)
                    ps = score_ps.tile([128, 512], f32, tag="score")
                    for ec in range(EC):
                        nc.tensor.matmul(
                            ps,
                            lhsT=qT_sb[:, ec, qsl],
                            rhs=kT_sb[:, kc // KCH, ec, kc % KCH, :],
                            start=(ec == 0),
                            stop=(ec == EC - 1),
                        )
                    nc.scalar.activation(
                        out=P_list[qt][:, csl],
                        in_=ps,
                        func=Act.Exp,
                        scale=inv_sqrt_d,
                        accum_out=den4_list[qt][:, kc : kc + 1],
                    )
                    if i >= 2:
                        emit_transposes(*units[i - 2])
                    if kc == KC - 1:
                        den = small.tile([128, 1], f32, tag="den", bufs=4)
                        nc.vector.reduce_sum(
                            out=den, in_=den4_list[qt], axis=mybir.AxisListType.X
                        )
                        nc.vector.reciprocal(recip_list[qt], den)
                emit_transposes(*units[-2])
                emit_transposes(*units[-1])

                # pass 2: attn + epilogue per q-tile
                for qt in range(QT):
                    qsl = slice(qt * 128, (qt + 1) * 128)
                    PT_sb = PT_list[qt]
                    recip = recip_list[qt]
                    xq_sb = att.tile([128, D], f32, tag="xq", bufs=3)
                    nc.sync.dma_start(out=xq_sb, in_=xq_d[qsl, :])
                    ot = att.tile([128, D], f32, tag="ot", bufs=3)
                    for j2 in range(EJ):
                        jsl = slice(j2 * 512, (j2 + 1) * 512)
                        pa = attn_ps.tile([128, 512], f32, tag="attn")
                        for j in range(SC):
                            nc.tensor.matmul(
                                pa,
                                lhsT=PT_sb[:, j, :],
                                rhs=v_sb[:, j // SCH, j % SCH, j2, :],
                                start=(j == 0),
                                stop=(j == SC - 1),
                            )
                        # out = attn * (1/den) + residual
                        nc.vector.scalar_tensor_tensor(
                            out=ot[:, jsl],
                            in0=pa,
                            scalar=recip,
                            in1=xq_sb[:, jsl],
                            op0=Alu.mult,
                            op1=Alu.add,
                        )
                        nc.scalar.dma_start(out=out_d[qsl, jsl], in_=ot[:, jsl])

    nc.compile()
    return nc


def _get_nc():
    if "nc" not in _cache:
        _cache["nc"] = _build()
    return _cache["nc"]


def kernel(embedded, Wq, bq, Wk, bk, Wv, bv):
    import ml_dtypes

    from concourse.bass_utils import run_bass_kernel_spmd

    bf16 = ml_dtypes.bfloat16
    x = np.ascontiguousarray(np.asarray(embedded, dtype=np.float32))
    Wq = np.asarray(Wq, dtype=np.float32)
    Wk = np.asarray(Wk, dtype=np.float32)
    Wv = np.asarray(Wv, dtype=np.float32)
    bq = np.ascontiguousarray(np.asarray(bq, dtype=np.float32))
    bk = np.ascontiguousarray(np.asarray(bk, dtype=np.float32))
    bv = np.ascontiguousarray(np.asarray(bv, dtype=np.float32))

    # e-chunk-major weight layouts: wE[ec, p, dc, j] = W.T[dc*128+p, ec*128+j]
    def echunk(wT):
        return np.ascontiguousarray(
            wT.reshape(DC, 128, EC, 128).transpose(2, 1, 0, 3)
        )

    wqT = np.ascontiguousarray(Wq.T).astype(bf16)
    wkT = np.ascontiguousarray(Wk.T).astype(bf16)
    wvT = np.ascontiguousarray(Wv.T).astype(bf16)
    wqE = echunk(wqT)
    wkE = echunk(wkT)
    xT = [np.ascontiguousarray(x[b].T).astype(bf16) for b in range(B)]

    in_maps = []
    for c in range(NCORES):
        b, h = c // 2, c % 2
        qs = slice(h * SQ, (h + 1) * SQ)
        in_maps.append(
            {
                "xqT": np.ascontiguousarray(xT[b][:, qs]),
                "xq": np.ascontiguousarray(x[b, qs, :] + bv),
                "wqE": wqE,
                "wkE": wkE,
                "wvT": wvT,
                "bq": bq,
            }
        )

    _cache["in_maps"] = in_maps
    nc = _get_nc()
    res = run_bass_kernel_spmd(nc, in_maps, core_ids=list(range(NCORES)))
    out = np.empty((B, S, D), dtype=np.float32)
    for c in range(NCORES):
        b, h = c // 2, c % 2
        out[b, h * SQ : (h + 1) * SQ, :] = res.results[c]["out"]
    return out


# revision 8
# speedup vs baseline: 1.5311x; 1.1671x over previous
"""Distributed single-head attention block for trn2 (8 NeuronCores), v3.

reference:
    q = x @ Wq.T + bq ; k = x @ Wk.T + bk ; v = x @ Wv.T + bv
    out = x + softmax(q @ k.T / sqrt(D)) @ v       x: [4, 2048, 1024]

Sharding: 8 cores = 4 batches x 2 halves. Core c owns batch c//2 and
rows [h*1024, (h+1)*1024) with h = c%2 — both as queries and as keys.
Each core projects Q, K and V only for its OWN half (perfectly
balanced: 3 x 1024^3 MACs of projection per core vs 4 x 1024^3 in the
K-replicated variant) and the pair exchanges K/V halves with two
pairwise AllGathers (2MB each), landing in natural batch order.

Engine-queue discipline (learned from v2's trace): a dma_start costs
~0.7us of ISSUE time on the triggering engine's in-order queue, so DMA
triggers must never sit ahead of latency-critical compute on the same
queue, and a trigger that waits on a collective poisons everything
behind it:
    sync   : weight/activation input loads, then K/V slot-0 readbacks,
             then residual (xq) loads
    scalar : bq load + K/V stage-outs (early), exp activations, output
             stores
    vector : psum->SBUF copies/casts and all other DVE math only
    gpsimd : collective triggers, then K/V slot-1 readbacks
Scores wait only on the K readbacks (~91us, PE is busy until ~89us);
attention waits on the V readbacks (~121us, PE busy until ~150us).

Device-side layouts (host pre-transposes + bf16-casts so the
contraction dim always lands on SBUF partitions):
    xqT  [D, SQ]            bf16  x[b, half].T   -> Q/K/V projections
    xq   [SQ, D]            f32   x[b, half]+bv  -> residual add
    wvT  [D, D]             bf16  Wv.T
    wkE  [EC, 128, DC, 128] bf16  Wk.T e-chunk-major (wkE[ec,p,dc,j]
          = Wk.T[dc*128+p, ec*128+j]) so the first kT chain only needs
          xqT + one 256KB e-chunk: first real matmul at ~7us.
    wqE  same for Wq.T
Projections emit qT/kT [e, s] (scores contraction over e) and v [s, e]
(attn contraction over keys). Softmax rows live on partitions: exp on
ScalarE with accum_out giving row sums for free; no max subtraction
(scores are O(10) for this model so exp cannot overflow in f32). P is
transposed 128x128 on TensorE (identity matmul) for the attn matmul;
transposes are emitted TWO score-chains late so the PE never waits on
the exp that produces their input. Score order is slot-0 chunks for
all q-tiles first (own-slot readback lands first), then kc in {2,3}
per q-tile so each tile's softmax denominator (and its reciprocal) is
ready long before the attention epilogue needs it. Only the Q bias is
applied on-device: the K bias cancels in softmax (per-row constant),
and the V bias is folded into the residual input on the host
(attention weights sum to 1).
"""

import numpy as np

B, S, D = 4, 2048, 1024
SQ = S // 2  # queries/keys owned per core
NCORES = 8
DC = D // 128  # contraction chunks
EC = D // 128  # embed chunks
SC = S // 128  # key chunks, full batch
SCH = SQ // 128  # key chunks per half
QT = SQ // 128  # query tiles per core
KC = S // 512  # score column chunks, full batch
KCH = SQ // 512  # score column chunks per half
EJ = D // 512  # 512-wide embed column chunks

_cache = {}


def _build():
    import concourse.bass as bass
    import concourse.tile as tile
    from concourse import bacc, mybir
    from concourse.masks import make_identity

    f32 = mybir.dt.float32
    bf16 = mybir.dt.bfloat16
    Alu = mybir.AluOpType
    Act = mybir.ActivationFunctionType

    nc = bacc.Bacc(None, target_bir_lowering=False, debug=False)

    xqT_d = nc.declare_dram_parameter("xqT", [D, SQ], bf16, isOutput=False)
    xq_d = nc.declare_dram_parameter("xq", [SQ, D], f32, isOutput=False)
    wqE_d = nc.declare_dram_parameter("wqE", [EC, 128, DC, 128], bf16, isOutput=False)
    wkE_d = nc.declare_dram_parameter("wkE", [EC, 128, DC, 128], bf16, isOutput=False)
    wv_d = nc.declare_dram_parameter("wvT", [D, D], bf16, isOutput=False)
    bq_d = nc.declare_dram_parameter("bq", [D], f32, isOutput=False)
    out_d = nc.declare_dram_parameter("out", [SQ, D], f32, isOutput=True)

    # Pairwise K/V exchange staging; AllGather slot order = natural
    # batch order, identical on both pair members (uniform SPMD graph).
    kx_in = nc.dram_tensor("kx_in", [EC, KCH, 128, 512], bf16)
    kx_out = nc.dram_tensor("kx_out", [2, EC, KCH, 128, 512], bf16)
    vx_in = nc.dram_tensor("vx_in", [SCH, EJ, 128, 512], bf16)
    vx_out = nc.dram_tensor("vx_out", [2, SCH, EJ, 128, 512], bf16)

    groups = [[0, 1], [2, 3], [4, 5], [6, 7]]

    with tile.TileContext(nc) as tc:
        with tc.tile_pool(name="pers", bufs=1) as pers:
            qT_sb = pers.tile([128, EC, SQ], bf16, tag="qT")
            # AG-fed K/V: [slot, ec|sc, col-chunk, 512]
            kT_sb = pers.tile([128, 2, EC, KCH, 512], bf16, tag="kT")
            v_sb = pers.tile([128, 2, SCH, EJ, 512], bf16, tag="v")
            ident = pers.tile([128, 128], bf16, tag="ident")
            make_identity(nc, ident)
            bq_sb = pers.tile([128, EC], f32, tag="bq")

            # PE warmup: dense dummy matmuls while the first input DMAs land,
            # so the HAM clock gate is already at 2.4GHz when real work starts.
            warm_sb = pers.tile([128, 512], bf16, tag="warm")
            warm_dump = pers.tile([128, 512], f32, tag="warm_dump")
            nc.vector.memset(warm_sb, 0.0)
            with tc.tile_pool(name="warm_ps", bufs=1, space="PSUM") as warm_ps:
                wps = warm_ps.tile([128, 512], f32, tag="wps")
                NWARM = 14
                for i in range(NWARM):
                    nc.tensor.matmul(
                        wps,
                        lhsT=warm_sb[:, 0:128],
                        rhs=warm_sb,
                        start=(i == 0),
                        stop=(i == NWARM - 1),
                    )
                nc.vector.tensor_copy(out=warm_dump, in_=wps)

            bq_ap = bq_d.ap()
            nc.scalar.dma_start(
                out=bq_sb,
                in_=bass.AP(tensor=bq_ap.tensor, offset=0, ap=[[1, 128], [128, EC]]),
            )

            with (
                tc.tile_pool(name="ld", bufs=1) as ld,
                tc.tile_pool(name="stage", bufs=4) as stage,
                tc.tile_pool(name="proj_ps", bufs=4, space="PSUM") as proj_ps,
            ):
                xqT_sb = ld.tile([128, DC, SQ], bf16, tag="xqT")
                wk_sb = ld.tile([128, EC, DC, 128], bf16, tag="wk")
                wq_sb = ld.tile([128, EC, DC, 128], bf16, tag="wq")
                wv_sb = ld.tile([128, DC, D], bf16, tag="wv")

                # DMA priority: xqT gates every projection; wk e-chunks gate
                # the kT chains (and with them the K exchange); then wv, wq.
                for dc in range(DC):
                    r = slice(dc * 128, (dc + 1) * 128)
                    nc.sync.dma_start(out=xqT_sb[:, dc, :], in_=xqT_d[r, :])
                for ec in range(EC):
                    nc.sync.dma_start(out=wk_sb[:, ec, :, :], in_=wkE_d[ec])
                for dc in range(DC):
                    r = slice(dc * 128, (dc + 1) * 128)
                    nc.sync.dma_start(out=wv_sb[:, dc, :], in_=wv_d[r, :])
                for ec in range(EC):
                    nc.sync.dma_start(out=wq_sb[:, ec, :, :], in_=wqE_d[ec])

                # kT own half [e, sk_own] -> bf16 stage tiles -> DRAM
                for ec in range(EC):
                    for kc in range(KCH):
                        csl = slice(kc * 512, (kc + 1) * 512)
                        ps = proj_ps.tile([128, 512], f32, tag="ps")
                        for dc in range(DC):
                            nc.tensor.matmul(
                                ps,
                                lhsT=wk_sb[:, ec, dc, :],
                                rhs=xqT_sb[:, dc, csl],
                                start=(dc == 0),
                                stop=(dc == DC - 1),
                            )
                        kst = stage.tile([128, 512], bf16, tag="kst")
                        nc.vector.tensor_copy(out=kst, in_=ps)
                        nc.scalar.dma_start(out=kx_in[ec, kc], in_=kst)
                nc.gpsimd.collective_compute(
                    "AllGather",
                    Alu.bypass,
                    replica_groups=groups,
                    ins=[kx_in.ap().opt()],
                    outs=[kx_out.ap().opt()],
                )
                # Readbacks: 2 big strided DMAs per tensor slot (cheap to
                # issue), on the sync queue which is idle after the input
                # loads. The scheduler places instructions as early as their
                # deps allow, so these sit right behind their collective;
                # gpsimd carries ONLY collective triggers (a blocked DMA at
                # the gpsimd queue head stalls the CC handshake machinery).
                kx_out_ap = kx_out.ap()
                for s in range(2):
                    for kc in range(KCH):
                        nc.sync.dma_start(
                            out=kT_sb[:, s, :, kc, :],
                            in_=bass.AP(
                                tensor=kx_out_ap.tensor,
                                offset=s * (EC * KCH * 128 * 512) + kc * (128 * 512),
                                ap=[[512, 128], [KCH * 128 * 512, EC], [1, 512]],
                            ),
                        )

                # v own half [sk_own, e], same pattern.
                for sc in range(SCH):
                    for j in range(EJ):
                        jsl = slice(j * 512, (j + 1) * 512)
                        ps = proj_ps.tile([128, 512], f32, tag="ps")
                        for dc in range(DC):
                            nc.tensor.matmul(
                                ps,
                                lhsT=xqT_sb[:, dc, sc * 128 : (sc + 1) * 128],
                                rhs=wv_sb[:, dc, jsl],
                                start=(dc == 0),
                                stop=(dc == DC - 1),
                            )
                        vst = stage.tile([128, 512], bf16, tag="vst")
                        nc.vector.tensor_copy(out=vst, in_=ps)
                        nc.scalar.dma_start(out=vx_in[sc, j], in_=vst)
                nc.gpsimd.collective_compute(
                    "AllGather",
                    Alu.bypass,
                    replica_groups=groups,
                    ins=[vx_in.ap().opt()],
                    outs=[vx_out.ap().opt()],
                )
                vx_out_ap = vx_out.ap()
                for s in range(2):
                    for j in range(EJ):
                        nc.sync.dma_start(
                            out=v_sb[:, s, :, j, :],
                            in_=bass.AP(
                                tensor=vx_out_ap.tensor,
                                offset=s * (SCH * EJ * 128 * 512) + j * (128 * 512),
                                ap=[[512, 128], [EJ * 128 * 512, SCH], [1, 512]],
                            ),
                        )

                # qT[e, sq] = sum_d wqT[d, e] * xqT[d, sq]  (+bq per-partition)
                for j in range(SQ // 512):
                    jsl = slice(j * 512, (j + 1) * 512)
                    for ec in range(EC):
                        ps = proj_ps.tile([128, 512], f32, tag="ps")
                        for dc in range(DC):
                            nc.tensor.matmul(
                                ps,
                                lhsT=wq_sb[:, ec, dc, :],
                                rhs=xqT_sb[:, dc, jsl],
                                start=(dc == 0),
                                stop=(dc == DC - 1),
                            )
                        nc.vector.tensor_scalar_add(
                            out=qT_sb[:, ec, jsl],
                            in0=ps,
                            scalar1=bq_sb[:, ec : ec + 1],
                        )

            with (
                tc.tile_pool(name="att", bufs=2) as att,
                tc.tile_pool(name="small", bufs=2) as small,
                tc.tile_pool(name="score_ps", bufs=3, space="PSUM") as score_ps,
                tc.tile_pool(name="tr_ps", bufs=3, space="PSUM") as tr_ps,
                tc.tile_pool(name="attn_ps", bufs=2, space="PSUM") as attn_ps,
            ):
                inv_sqrt_d = float(1.0 / np.sqrt(D))
                P_list = [
                    att.tile([128, S], bf16, name=f"P{qt}", tag=f"P{qt}", bufs=1)
                    for qt in range(QT)
                ]
                PT_list = [
                    att.tile(
                        [128, SC, 128], bf16, name=f"PT{qt}", tag=f"PT{qt}", bufs=1
                    )
                    for qt in range(QT)
                ]
                den4_list = [
                    small.tile([128, KC], f32, name=f"den4{qt}", tag=f"den4{qt}", bufs=1)
                    for qt in range(QT)
                ]
                recip_list = [
                    small.tile(
                        [128, 1], f32, name=f"recip{qt}", tag=f"recip{qt}", bufs=1
                    )
                    for qt in range(QT)
                ]

                # pass 1: scores -> exp -> P transposes. Unit order: slot-0
                # key chunks across all q-tiles first (their readback lands
                # first), then kc in {2,3} per q-tile so den/recip per tile
                # completes well before the attn epilogue. Transposes for a
                # unit are emitted two chains late so the PE never waits on
                # that unit's exp.
                units = [(kc, qt) for kc in range(KCH) for qt in range(QT)]
                units += [(kc, qt) for qt in range(QT) for kc in range(KCH, KC)]

                def emit_transposes(kc, qt):
                    for j in range(kc * 4, kc * 4 + 4):
                        tp = tr_ps.tile([128, 128], bf16, tag="tr")
                        nc.tensor.transpose(
                            tp, P_list[qt][:, j * 128 : (j + 1) * 128], ident
                        )
                        nc.vector.tensor_copy(out=PT_list[qt][:, j, :], in_=tp)

                for i, (kc, qt) in enumerate(units):
                    csl = slice(kc * 512, (kc + 1) * 512)
                    qsl = slice(qt * 128, (qt + 1) * 128)
                    ps = score_ps.tile([128, 512], f32, tag="score")
                    for ec in range(EC):
                        nc.tensor.matmul(
                            ps,
                            lhsT=qT_sb[:, ec, qsl],
                            rhs=kT_sb[:, kc // KCH, ec, kc % KCH, :],
                            start=(ec == 0),
                            stop=(ec == EC - 1),
                        )
                    nc.scalar.activation(
                        out=P_list[qt][:, csl],
                        in_=ps,
                        func=Act.Exp,
                        scale=inv_sqrt_d,
                        accum_out=den4_list[qt][:, kc : kc + 1],
                    )
                    if i >= 2:
                        emit_transposes(*units[i - 2])
                    if kc == KC - 1:
                        den = small.tile([128, 1], f32, tag="den", bufs=4)
                        nc.vector.reduce_sum(
                            out=den, in_=den4_list[qt], axis=mybir.AxisListType.X
                        )
                        nc.vector.reciprocal(recip_list[qt], den)
                emit_transposes(*units[-2])
                emit_transposes(*units[-1])

                # pass 2: attn + epilogue per q-tile
                for qt in range(QT):
                    qsl = slice(qt * 128, (qt + 1) * 128)
                    PT_sb = PT_list[qt]
                    recip = recip_list[qt]
                    xq_sb = att.tile([128, D], f32, tag="xq", bufs=3)
                    nc.sync.dma_start(out=xq_sb, in_=xq_d[qsl, :])
                    ot = att.tile([128, D], f32, tag="ot", bufs=3)
                    for j2 in range(EJ):
                        jsl = slice(j2 * 512, (j2 + 1) * 512)
                        pa = attn_ps.tile([128, 512], f32, tag="attn")
                        for j in range(SC):
                            nc.tensor.matmul(
                                pa,
                                lhsT=PT_sb[:, j, :],
                                rhs=v_sb[:, j // SCH, j % SCH, j2, :],
                                start=(j == 0),
                                stop=(j == SC - 1),
                            )
                        # out = attn * (1/den) + residual
                        nc.vector.scalar_tensor_tensor(
                            out=ot[:, jsl],
                            in0=pa,
                            scalar=recip,
                            in1=xq_sb[:, jsl],
                            op0=Alu.mult,
                            op1=Alu.add,
                        )
                        nc.scalar.dma_start(out=out_d[qsl, jsl], in_=ot[:, jsl])

    nc.compile()
    return nc


def _get_nc():
    if "nc" not in _cache:
        _cache["nc"] = _build()
    return _cache["nc"]


def kernel(embedded, Wq, bq, Wk, bk, Wv, bv):
    import ml_dtypes

    from concourse.bass_utils import run_bass_kernel_spmd

    bf16 = ml_dtypes.bfloat16
    x = np.ascontiguousarray(np.asarray(embedded, dtype=np.float32))
    Wq = np.asarray(Wq, dtype=np.float32)
    Wk = np.asarray(Wk, dtype=np.float32)
    Wv = np.asarray(Wv, dtype=np.float32)
    bq = np.ascontiguousarray(np.asarray(bq, dtype=np.float32))
    bk = np.ascontiguousarray(np.asarray(bk, dtype=np.float32))
    bv = np.ascontiguousarray(np.asarray(bv, dtype=np.float32))

    # e-chunk-major weight layouts: wE[ec, p, dc, j] = W.T[dc*128+p, ec*128+j]
    def echunk(wT):
        return np.ascontiguousarray(
            wT.reshape(DC, 128, EC, 128).transpose(2, 1, 0, 3)
        )

    wqT = np.ascontiguousarray(Wq.T).astype(bf16)
    wkT = np.ascontiguousarray(Wk.T).astype(bf16)
    wvT = np.ascontiguousarray(Wv.T).astype(bf16)
    wqE = echunk(wqT)
    wkE = echunk(wkT)
    xT = [np.ascontiguousarray(x[b].T).astype(bf16) for b in range(B)]

    in_maps = []
    for c in range(NCORES):
        b, h = c // 2, c % 2
        qs = slice(h * SQ, (h + 1) * SQ)
        in_maps.append(
            {
                "xqT": np.ascontiguousarray(xT[b][:, qs]),
                "xq": np.ascontiguousarray(x[b, qs, :] + bv),
                "wqE": wqE,
                "wkE": wkE,
                "wvT": wvT,
                "bq": bq,
            }
        )

    _cache["in_maps"] = in_maps
    nc = _get_nc()
    res = run_bass_kernel_spmd(nc, in_maps, core_ids=list(range(NCORES)))
    out = np.empty((B, S, D), dtype=np.float32)
    for c in range(NCORES):
        b, h = c // 2, c % 2
        out[b, h * SQ : (h + 1) * SQ, :] = res.results[c]["out"]
    return out
